# revision 9
# baseline (speedup 1.0000x reference)
"""MultiHeadAttention Trainium2 kernel (8 NeuronCores).

Sharding: data-parallel over batch (2) x tensor-parallel over heads (4 per
core). Core c handles batch b = c//4, heads 4g..4g+3 where g = c%4.
Per-core device kernel computes:
  - qT/kT (head-transposed projections) and v (natural) for its 4 heads
  - scores in both [q,k] (for the attn output + row sums) and [k,q]
    (for the attn @ v product) orientations; exp on the Scalar engine
  - attn (normalized, f32) written to DRAM; ctxT = (exp @ v).T normalized
  - fc partial = ctxT.T @ WfcT_local (+ bfc on g==0), ReduceScattered over
    the 4 cores of the batch group so rank g ends with output rows
    512g..512g+512.
All matmuls run as float32r (rounded fp32, 1 cycle/column at N>=256).
"""

import sys

if "/opt/trn_rl_repo" not in sys.path:
    sys.path.insert(0, "/opt/trn_rl_repo")

import numpy as np
from contextlib import ExitStack

import concourse.bass as bass
import concourse.tile as tile
from concourse import bacc, mybir
from concourse.bass_utils import run_bass_kernel_spmd
from concourse.masks import make_identity

F32 = mybir.dt.float32
F32R = mybir.dt.float32r

N_CORES = 8
B = 2
S = 2048
D = 1024
H_TOTAL = 16
DK = 64
H_LOC = 4          # heads per core
JL = H_LOC * DK    # 256 local projection width
NT = S // 128      # 16 s-tiles
NC4 = S // 512     # 4 s-chunks
KC = D // 128      # 8 d-chunks

AF = mybir.ActivationFunctionType
ALU = mybir.AluOpType
AX = mybir.AxisListType


def build_kernel() -> bass.Bass:
    nc = bacc.Bacc("TRN2", target_bir_lowering=False, debug=False,
                   num_devices=N_CORES)

    xq = nc.dram_tensor("xq", [S, D], F32, kind="ExternalInput")
    xk = nc.dram_tensor("xk", [S, D], F32, kind="ExternalInput")
    xv = nc.dram_tensor("xv", [S, D], F32, kind="ExternalInput")
    wq = nc.dram_tensor("wq", [JL, D], F32, kind="ExternalInput")
    wk = nc.dram_tensor("wk", [JL, D], F32, kind="ExternalInput")
    wv = nc.dram_tensor("wv", [JL, D], F32, kind="ExternalInput")
    wfc = nc.dram_tensor("wfc", [D, JL], F32, kind="ExternalInput")
    bqt = nc.dram_tensor("bqt", [128, 2], F32, kind="ExternalInput")
    bkt = nc.dram_tensor("bkt", [128, 2], F32, kind="ExternalInput")
    bv_d = nc.dram_tensor("bv_d", [1, JL], F32, kind="ExternalInput")
    bfc_d = nc.dram_tensor("bfc_d", [1, D], F32, kind="ExternalInput")

    attn_out = nc.dram_tensor("attn_out", [H_LOC, S, S], F32,
                              kind="ExternalOutput")
    out_slice = nc.dram_tensor("out_slice", [S // 4, D], F32,
                               kind="ExternalOutput")

    with ExitStack() as ctx:
        tc = ctx.enter_context(tile.TileContext(nc))

        const = ctx.enter_context(tc.tile_pool(name="const", bufs=1))
        wnat_p = ctx.enter_context(tc.tile_pool(name="wnat", bufs=1))
        wt_p = ctx.enter_context(tc.tile_pool(name="wt", bufs=3))
        bias_p = ctx.enter_context(tc.tile_pool(name="bias", bufs=1))
        xnat_p = ctx.enter_context(tc.tile_pool(name="xnat", bufs=2))
        xt_p = ctx.enter_context(tc.tile_pool(name="xt", bufs=10))
        qk_p = ctx.enter_context(tc.tile_pool(name="qk", bufs=1))
        v_p = ctx.enter_context(tc.tile_pool(name="vp", bufs=16))
        exp_p = ctx.enter_context(tc.tile_pool(name="expp", bufs=2))
        expt_p = ctx.enter_context(tc.tile_pool(name="exptp", bufs=3))
        ctx_p = ctx.enter_context(tc.tile_pool(name="ctxp", bufs=1))
        sums_p = ctx.enter_context(tc.tile_pool(name="sumsp", bufs=4))
        small_p = ctx.enter_context(tc.tile_pool(name="smallp", bufs=4))
        rt_p = ctx.enter_context(tc.tile_pool(name="rtp", bufs=2))
        fc_p = ctx.enter_context(tc.tile_pool(name="fcp", bufs=2))

        pa = ctx.enter_context(tc.tile_pool(name="pa", bufs=2, space="PSUM"))
        pb = ctx.enter_context(tc.tile_pool(name="pb", bufs=2, space="PSUM"))
        pc = ctx.enter_context(tc.tile_pool(name="pc", bufs=2, space="PSUM"))
        pt = ctx.enter_context(tc.tile_pool(name="pt", bufs=2, space="PSUM"))

        dram = ctx.enter_context(tc.tile_pool(name="dram", bufs=1, space="DRAM"))

        ident = const.tile([128, 128], F32, tag="ident")
        make_identity(nc, ident[:])
        ones1f = const.tile([1, 128], F32, tag="ones1f")
        nc.gpsimd.memset(ones1f[:], 1.0)
        ones1 = const.tile([1, 128], F32R, tag="ones1")
        nc.vector.tensor_copy(ones1[:], ones1f[:])

        # ---- biases ----
        bqt_sb = bias_p.tile([128, 2], F32, tag="bqt")
        nc.sync.dma_start(bqt_sb[:], bqt[:])
        bkt_sb = bias_p.tile([128, 2], F32, tag="bkt")
        nc.sync.dma_start(bkt_sb[:], bkt[:])
        bv_sb = bias_p.tile([1, JL], F32R, tag="bv")
        nc.gpsimd.dma_start(bv_sb[:], bv_d[:])
        bfc_sb = bias_p.tile([1, D], F32R, tag="bfc")
        nc.gpsimd.dma_start(bfc_sb[:], bfc_d[:])

        # ---- weight transposes: wXT[d-chunk kc partitions, kc*256 + j] ----
        def load_wT(w_dram, name):
            wT = wt_p.tile([128, KC * JL], F32R, name=name, tag="wT")
            for half in range(2):
                wn = wnat_p.tile([128, D], F32, name=f"{name}_nat", tag="wnat")
                nc.sync.dma_start(wn[:], w_dram[half * 128:(half + 1) * 128, :])
                for kc in range(KC):
                    ps = pt.tile([128, 128], F32, name=f"{name}_ps", tag="pt")
                    nc.tensor.transpose(ps[:], wn[:, kc * 128:(kc + 1) * 128],
                                        ident[:])
                    nc.vector.tensor_copy(
                        wT[:, kc * JL + half * 128: kc * JL + half * 128 + 128],
                        ps[:])
            return wT

        wqT = load_wT(wq, "wqT")
        wkT = load_wT(wk, "wkT")
        wvT = load_wT(wv, "wvT")

        # wfcT[p][64 c-of-head, half*1024 + o] from wfc [1024, 256]
        wfcT = []
        for p in range(2):
            t = wt_p.tile([64, 2 * D], F32R, name=f"wfcT{p}", tag=f"wfcT{p}", bufs=1)
            wfcT.append(t)
        for oc in range(KC):
            wn = wnat_p.tile([128, JL], F32, name="wfc_nat", tag="wfcnat")
            nc.sync.dma_start(wn[:], wfc[oc * 128:(oc + 1) * 128, :])
            for p in range(2):
                for half in range(2):
                    h = 2 * p + half
                    ps = pt.tile([64, 128], F32, name="wfc_ps", tag="pt")
                    nc.tensor.transpose(
                        ps[:], wn[:, h * 64:(h + 1) * 64], ident[:])
                    nc.vector.tensor_copy(
                        wfcT[p][:, half * D + oc * 128:
                                half * D + (oc + 1) * 128], ps[:])

        # ---- persistent activation tiles ----
        qT = [qk_p.tile([128, S], F32R, name=f"qT{p}", tag=f"qT{p}") for p in range(2)]
        kT = [qk_p.tile([128, S], F32R, name=f"kT{p}", tag=f"kT{p}") for p in range(2)]
        v_sb = [v_p.tile([128, JL], F32R, name=f"v{i}", tag="v") for i in range(NT)]
        ctxT = [ctx_p.tile([64, 2 * S], F32R, name=f"ctxT{p}", tag=f"ctxT{p}") for p in range(2)]

        # ---- input transposes + projections, per tensor ----
        def do_proj(x_dram, kind):
            # kind: "q" / "k" / "v"
            for n in range(NC4):
                xt = []
                for kc in range(KC):
                    t = xt_p.tile([128, 512], F32R, name=f"xt_{kind}", tag="xt")
                    xt.append(t)
                for i in range(4):
                    si = n * 4 + i
                    xn = xnat_p.tile([128, D], F32, name=f"xn_{kind}", tag="xn")
                    nc.sync.dma_start(xn[:], x_dram[si * 128:(si + 1) * 128, :])
                    for kc in range(KC):
                        ps = pt.tile([128, 128], F32, name="xt_ps", tag="pt")
                        nc.tensor.transpose(
                            ps[:], xn[:, kc * 128:(kc + 1) * 128], ident[:])
                        nc.vector.tensor_copy(
                            xt[kc][:, i * 128:(i + 1) * 128], ps[:])
                if kind in ("q", "k"):
                    wT = wqT if kind == "q" else wkT
                    bT = bqt_sb if kind == "q" else bkt_sb
                    dst = qT if kind == "q" else kT
                    for p in range(2):
                        ps = pc.tile([128, 512], F32, name="proj_ps", tag="pc")
                        for kc in range(KC):
                            nc.tensor.matmul(
                                ps[:],
                                wT[:, kc * JL + p * 128: kc * JL + (p + 1) * 128],
                                xt[kc][:],
                                start=(kc == 0), stop=(kc == KC - 1))
                        nc.vector.tensor_scalar_add(
                            dst[p][:, n * 512:(n + 1) * 512], ps[:],
                            bT[:, p:p + 1])
                else:
                    for i in range(4):
                        si = n * 4 + i
                        ps = pc.tile([128, JL], F32, name="projv_ps", tag="pc")
                        for kc in range(KC):
                            nc.tensor.matmul(
                                ps[:], xt[kc][:, i * 128:(i + 1) * 128],
                                wvT[:, kc * JL:(kc + 1) * JL],
                                start=(kc == 0), stop=False)
                        nc.tensor.matmul(ps[:], ones1[:], bv_sb[:],
                                         start=False, stop=True)
                        nc.vector.tensor_copy(v_sb[si][:], ps[:])

        do_proj(xq, "q")
        do_proj(xk, "k")
        do_proj(xv, "v")

        # ---- attention, per head pair ----
        for p in range(2):
            recip_row = []
            for half in range(2):
                h = 2 * p + half
                po = 64 * half
                sums_h = sums_p.tile([128, NT], F32, name=f"sums{h}",
                                     tag="sums")
                for i in range(NT):
                    exp_t = exp_p.tile([128, S], F32, name="exp_t", tag="exp")
                    sums4 = small_p.tile([128, 4], F32, name="sums4",
                                         tag="sums4")
                    for kc4 in range(4):
                        ps = pa.tile([128, 512], F32, name="sc_ps", tag="pa")
                        nc.tensor.matmul(
                            ps[:],
                            qT[p][po:po + 64, i * 128:(i + 1) * 128],
                            kT[p][po:po + 64, kc4 * 512:(kc4 + 1) * 512],
                            start=True, stop=True,
                            tile_position=(po, 0))
                        nc.scalar.activation(
                            exp_t[:, kc4 * 512:(kc4 + 1) * 512], ps[:],
                            AF.Exp, scale=0.125,
                            accum_out=sums4[:, kc4:kc4 + 1])
                    nc.vector.tensor_reduce(
                        sums_h[:, i:i + 1], sums4[:], axis=AX.X, op=ALU.add)
                    recip = small_p.tile([128, 1], F32, name="recip",
                                         tag="recip")
                    nc.vector.reciprocal(recip[:], sums_h[:, i:i + 1])
                    nc.vector.tensor_scalar_mul(exp_t[:], exp_t[:], recip[:])
                    nc.sync.dma_start(
                        attn_out[h, i * 128:(i + 1) * 128, :], exp_t[:])
                # 1/rowsum laid out along the free axis, for the AV normalize:
                # sums [128,16] -PE-T-> [16,128] -DRAM bounce-> [1,2048] -> 1/x
                ps_s = pt.tile([16, 128], F32, name="sumsT_ps", tag="pt")
                nc.tensor.transpose(ps_s[:], sums_h[:], ident[:])
                sT = small_p.tile([16, 128], F32, name="sT", tag="sT", bufs=2)
                nc.vector.tensor_copy(sT[:], ps_s[:])
                drow = dram.tile([1, S], F32, name=f"drow{h}", tag="drow",
                                 bufs=4)
                nc.sync.dma_start(
                    drow[0, :].rearrange("(a b) -> a b", a=16), sT[:])
                rrow = rt_p.tile([1, S], F32R, name=f"rrow{h}", tag="rrow")
                nc.gpsimd.dma_start(rrow[:], drow[:])
                with nc.allow_low_precision(reason="f32r is full fp32 width"):
                    nc.vector.reciprocal(rrow[:], rrow[:])
                recip_row.append(rrow)

            for n4 in range(NC4):
                ctxps = [pc.tile([64, 512], F32, name=f"ctx_ps{half}",
                                 tag="pc") for half in range(2)]
                for j in range(NT):
                    for half in range(2):
                        po = 64 * half
                        ps = pb.tile([128, 512], F32, name="scT_ps", tag="pb")
                        nc.tensor.matmul(
                            ps[:],
                            kT[p][po:po + 64, j * 128:(j + 1) * 128],
                            qT[p][po:po + 64, n4 * 512:(n4 + 1) * 512],
                            start=True, stop=True,
                            tile_position=(po, 0))
                        expT = expt_p.tile([128, 512], F32R, name="expT",
                                           tag="expT")
                        nc.scalar.activation(expT[:], ps[:], AF.Exp,
                                             scale=0.125)
                        h = 2 * p + half
                        nc.tensor.matmul(
                            ctxps[half][:],
                            v_sb[j][:, h * 64:(h + 1) * 64],
                            expT[:],
                            start=(j == 0), stop=(j == NT - 1))
                # broadcast 1/rowsum across partitions via rank-1 outer
                # product, then scale ctxT
                for half in range(2):
                    rb_ps = pt.tile([64, 512], F32, name="rb_ps", tag="pt")
                    nc.tensor.matmul(
                        rb_ps[:],
                        ones1[0:1, 0:64],
                        recip_row[half][0:1, n4 * 512:(n4 + 1) * 512],
                        start=True, stop=True)
                    rb_sb = small_p.tile([64, 512], F32, name="rb_sb",
                                         tag="rb", bufs=2)
                    nc.vector.tensor_copy(rb_sb[:], rb_ps[:])
                    nc.vector.tensor_tensor(
                        ctxT[p][:, half * S + n4 * 512:
                                half * S + (n4 + 1) * 512],
                        ctxps[half][:],
                        rb_sb[:],
                        op=ALU.mult)

        # ---- fc partial + ReduceScatter ----
        cc_in = dram.tile([S, D], F32)
        cc_out = dram.tile([S // 4, D], F32)
        for i in range(NT):
            for oc in range(2):
                fc_t = fc_p.tile([128, 512], F32, name="fc_t", tag="fc")
                ps = pc.tile([128, 512], F32, name="fc_ps", tag="pc")
                for p in range(2):
                    for half in range(2):
                        nc.tensor.matmul(
                            ps[:],
                            ctxT[p][:, half * S + i * 128:
                                    half * S + (i + 1) * 128],
                            wfcT[p][:, half * D + oc * 512:
                                    half * D + (oc + 1) * 512],
                            start=(p == 0 and half == 0), stop=False)
                nc.tensor.matmul(ps[:], ones1[:],
                                 bfc_sb[:, oc * 512:(oc + 1) * 512],
                                 start=False, stop=True)
                nc.vector.tensor_copy(fc_t[:], ps[:])
                nc.sync.dma_start(
                    cc_in[i * 128:(i + 1) * 128, oc * 512:(oc + 1) * 512],
                    fc_t[:])

        nc.gpsimd.collective_compute(
            "ReduceScatter",
            ALU.add,
            ins=[cc_in[:].opt()],
            outs=[cc_out[:].opt()],
            replica_groups=[[0, 1, 2, 3], [4, 5, 6, 7]],
        )
        nc.sync.dma_start(out_slice[:], cc_out[:])

    nc.compile()
    return nc


_NC_CACHE = None


def _get_nc():
    global _NC_CACHE
    if _NC_CACHE is None:
        _NC_CACHE = build_kernel()
    return _NC_CACHE


def kernel(Q, K, V, Wq, bq, Wk, bk, Wv, bv, Wfc, bfc):
    Q = np.asarray(Q, np.float32)
    K = np.asarray(K, np.float32)
    V = np.asarray(V, np.float32)
    Wq = np.asarray(Wq, np.float32)
    bq = np.asarray(bq, np.float32)
    Wk = np.asarray(Wk, np.float32)
    bk = np.asarray(bk, np.float32)
    Wv = np.asarray(Wv, np.float32)
    bv = np.asarray(bv, np.float32)
    Wfc = np.asarray(Wfc, np.float32)
    bfc = np.asarray(bfc, np.float32)

    nc = _get_nc()
    in_maps = []
    for c in range(N_CORES):
        b = c // 4
        g = c % 4
        sl = slice(JL * g, JL * (g + 1))
        in_maps.append({
            "xq": np.ascontiguousarray(Q[b]),
            "xk": np.ascontiguousarray(K[b]),
            "xv": np.ascontiguousarray(V[b]),
            "wq": np.ascontiguousarray(Wq[sl]),
            "wk": np.ascontiguousarray(Wk[sl]),
            "wv": np.ascontiguousarray(Wv[sl]),
            "wfc": np.ascontiguousarray(Wfc[:, sl]),
            "bqt": np.ascontiguousarray(bq[sl].reshape(2, 128).T),
            "bkt": np.ascontiguousarray(bk[sl].reshape(2, 128).T),
            "bv_d": np.ascontiguousarray(bv[sl].reshape(1, JL)),
            "bfc_d": np.ascontiguousarray(
                bfc.reshape(1, D) if g == 0 else np.zeros((1, D), np.float32)),
        })

    res = run_bass_kernel_spmd(nc, in_maps, core_ids=list(range(N_CORES)))

    out = np.empty((B, S, D), np.float32)
    attn = np.empty((B, H_TOTAL, S, S), np.float32)
    for c in range(N_CORES):
        b = c // 4
        g = c % 4
        attn[b, 4 * g:4 * (g + 1)] = res.results[c]["attn_out"]
        out[b, 512 * g:512 * (g + 1)] = res.results[c]["out_slice"]
    return out, attn
